# revision 2
# baseline (speedup 1.0000x reference)
"""Multi-head attention (B=8, S=1024, D=768, H=12, DH=64) on 8 TRN2 NeuronCores.

Strategy: pure data parallelism over batch — core b computes batch element b
end-to-end (no collectives). Per core, activations are kept in transposed
[feature, token] layout so every matmul contracts over the partition dim:

  xT [768,1024]  (host-transposed, bf16)
  qT/kT/vT per head-pair [128,1024] = Wqkv_pair.T @ xT   (PE, k=128, m=128)
  S^T per (pair, tchunk, shalf) [128,1024] = (even|odd) scores, k=64 row-tiled
  P = exp(S^T * 0.125)        (ACT, psum->sbuf bf16)
  O^T[65,512] += [V|1].T @ P  (PE; row 64 = softmax denominator for free)
  normalize: recip(denom) -> DRAM -> partition-broadcast DMA -> DVE multiply
  Y [1024,768] = OT.T @ Wo + bo  (PE k=128; DVE bias-add; natural layout out)

All matmul inputs bf16 (fp32 PSUM accumulation); output f32.
"""

import sys

sys.path.insert(0, "/opt/trn_rl_repo")

import numpy as np
import ml_dtypes

B, S, D = 8, 1024, 768
H = 12
DH = 64
NPAIR = 6  # head pairs
NDC = 6  # 128-wide chunks of D
NTC = 8  # 128-wide chunks of S (key/t side)
NSC = 8  # 128-wide chunks of S (query/s side)

_BF16 = ml_dtypes.bfloat16

_cache = {}


def _build_program():
    import concourse.bass as bass
    import concourse.bacc as bacc
    import concourse.tile as tile
    from concourse import mybir

    F32 = mybir.dt.float32
    BF16 = mybir.dt.bfloat16
    Exp = mybir.ActivationFunctionType.Exp

    nc = bacc.Bacc("TRN2", target_bir_lowering=False, debug=False)

    # ---- DRAM I/O (per core) ----
    xT_d = nc.dram_tensor("xT", [D, S], BF16, kind="ExternalInput")
    wq_d = nc.dram_tensor("wq", [NPAIR, 128, D], BF16, kind="ExternalInput")
    wk_d = nc.dram_tensor("wk", [NPAIR, 128, D], BF16, kind="ExternalInput")
    wv_d = nc.dram_tensor("wv", [NPAIR, 128, D], BF16, kind="ExternalInput")
    wo_d = nc.dram_tensor("wo", [NDC, 128, D], BF16, kind="ExternalInput")
    bqkv_d = nc.dram_tensor("bqkv", [128, 18], F32, kind="ExternalInput")
    bo_d = nc.dram_tensor("bo", [1, D], F32, kind="ExternalInput")
    ident_d = nc.dram_tensor("ident", [128, 64], BF16, kind="ExternalInput")
    y_d = nc.dram_tensor("y", [S, D], F32, kind="ExternalOutput")

    denom_d = nc.dram_tensor("denom_scr", [H, S], F32, kind="Internal")

    with tile.TileContext(nc) as tc:
        import contextlib

        ctx = contextlib.ExitStack()
        with ctx:
            const = ctx.enter_context(tc.tile_pool(name="const", bufs=1))
            wpool = ctx.enter_context(tc.tile_pool(name="wpool", bufs=1))
            qkv = ctx.enter_context(tc.tile_pool(name="qkv", bufs=2))
            vn_pool = ctx.enter_context(tc.tile_pool(name="vn", bufs=4))
            ot_pool = ctx.enter_context(tc.tile_pool(name="ot", bufs=1))
            e_pool = ctx.enter_context(tc.tile_pool(name="e", bufs=3))
            r_pool = ctx.enter_context(tc.tile_pool(name="r", bufs=4))
            y_pool = ctx.enter_context(tc.tile_pool(name="ysb", bufs=2))
            ps = ctx.enter_context(tc.tile_pool(name="ps", bufs=1, space="PSUM"))

            # ---- constants / weights resident in SBUF ----
            ident = const.tile([128, 64], BF16)
            nc.sync.dma_start(ident, ident_d[:, :])
            bqkv = const.tile([128, 18], F32)
            nc.sync.dma_start(bqkv, bqkv_d[:, :])
            bo_b = const.tile([128, D], F32)
            nc.sync.dma_start(
                bo_b, bass.AP(tensor=bo_d, offset=0, ap=[[0, 128], [1, D]])
            )

            xT = []
            for dc in range(NDC):
                t = wpool.tile([128, S], BF16, name=f"xT{dc}")
                nc.sync.dma_start(t, xT_d[dc * 128 : (dc + 1) * 128, :])
                xT.append(t)

            w_sb = {}
            for wname, wd in (("q", wq_d), ("k", wk_d), ("v", wv_d)):
                for p in range(NPAIR):
                    t = wpool.tile([128, D], BF16, name=f"w{wname}{p}")
                    # sbuf [d_sub, dc*128+m] <- dram [p, m, dc*128+d_sub]:
                    # host packs as [pair, 128(d_sub), 768(dc*128+m)] already.
                    nc.sync.dma_start(t, wd[p, :, :])
                    w_sb[wname, p] = t
            wo_sb = []
            for dc in range(NDC):
                t = wpool.tile([128, D], BF16, name=f"wo{dc}")
                nc.sync.dma_start(t, wo_d[dc, :, :])
                wo_sb.append(t)

            # persistent OT tiles (one per pair, [128, 1024] bf16)
            OT = [ot_pool.tile([128, S], BF16, name=f"OT{p}") for p in range(NPAIR)]

            def projection(p, wname, bias_col):
                """Compute (W_pair.T @ xT + b) -> bf16 [128, 1024] tile."""
                pst = ps.tile([128, S], F32, tag="pp", bufs=1, name=f"pp_{wname}{p}")
                w = w_sb[wname, p]
                for dc in range(NDC):
                    lhsT = w[:, dc * 128 : (dc + 1) * 128]
                    for nh in range(2):
                        nc.tensor.matmul(
                            pst[:, nh * 512 : (nh + 1) * 512],
                            lhsT,
                            xT[dc][:, nh * 512 : (nh + 1) * 512],
                            start=(dc == 0),
                            stop=(dc == NDC - 1),
                        )
                dst = qkv.tile([128, S], BF16, tag=wname, name=f"{wname}T{p}")
                nc.vector.tensor_scalar_add(dst, pst, bqkv[:, bias_col : bias_col + 1])
                return dst

            for p in range(NPAIR):
                qT = projection(p, "q", 0 * 6 + p)
                kT = projection(p, "k", 1 * 6 + p)
                vT = projection(p, "v", 2 * 6 + p)

                # V natural [t, e] per head, packed as 8 blocks of [128, 64+1(ones)]
                vnat = []
                for h2 in range(2):
                    vps = ps.tile([128, 512], BF16, tag="pp", bufs=1, name=f"vn{p}_{h2}")
                    for tcb in range(NTC):
                        nc.tensor.transpose(
                            vps[:, tcb * 64 : (tcb + 1) * 64],
                            vT[h2 * 64 : (h2 + 1) * 64, tcb * 128 : (tcb + 1) * 128],
                            ident[h2 * 64 : (h2 + 1) * 64, :],
                        )
                    vn = vn_pool.tile([128, NTC * 65], BF16, name=f"vnat{p}_{h2}")
                    vn_r = vn.rearrange("a (b c) -> a b c", c=65)
                    nc.vector.tensor_copy(
                        vn_r[:, :, 0:64], vps.rearrange("a (b c) -> a b c", c=64)
                    )
                    nc.vector.memset(vn_r[:, :, 64:65], 1.0)
                    vnat.append(vn)

                # attention core; s split in halves to fit PSUM
                for sh in range(2):
                    ssl = slice(sh * 512, (sh + 1) * 512)
                    Ops = [
                        ps.tile([65, 512], F32, tag="o", bufs=2, name=f"o{p}_{sh}_{h2}")
                        for h2 in range(2)
                    ]
                    for tcb in range(NTC):
                        st = ps.tile([128, S], F32, tag="s", bufs=2, name=f"s{p}_{sh}_{tcb}")
                        nc.tensor.matmul(
                            st[:, 0:512],
                            kT[0:64, tcb * 128 : (tcb + 1) * 128],
                            qT[0:64, ssl],
                            start=True,
                            stop=True,
                        )
                        nc.tensor.matmul(
                            st[:, 512:1024],
                            kT[64:128, tcb * 128 : (tcb + 1) * 128],
                            qT[64:128, ssl],
                            start=True,
                            stop=True,
                        )
                        et = e_pool.tile([128, S], BF16, name="expS")
                        nc.scalar.activation(et, st, Exp, scale=0.125)
                        for h2 in range(2):
                            nc.tensor.matmul(
                                Ops[h2][:, :],
                                vnat[h2][:, tcb * 65 : (tcb + 1) * 65],
                                et[:, h2 * 512 : (h2 + 1) * 512],
                                start=(tcb == 0),
                                stop=(tcb == NTC - 1),
                            )
                    # normalize both heads of this s-half
                    for h2 in range(2):
                        h = 2 * p + h2
                        rt = r_pool.tile([65, 512], F32, tag="rt", name="rt")
                        nc.vector.reciprocal(out=rt[64:65, :], in_=Ops[h2][64:65, :])
                        nc.sync.dma_start(denom_d[h, ssl], rt[64:65, :])
                        rb = r_pool.tile([64, 512], F32, tag="rb", name="rb")
                        nc.sync.dma_start(
                            rb,
                            bass.AP(
                                tensor=denom_d,
                                offset=h * S + sh * 512,
                                ap=[[0, 64], [1, 512]],
                            ),
                        )
                        nc.vector.tensor_mul(
                            OT[p][h2 * 64 : (h2 + 1) * 64, ssl], Ops[h2][0:64, :], rb
                        )

            # ---- output projection: Y[sc] = sum_dc OT[dc][:, sc].T @ Wo[dc] + bo
            for sc in range(NSC):
                yps = ps.tile([128, D], F32, tag="s", bufs=2, name=f"y{sc}")
                for dc in range(NDC):
                    lhsT = OT[dc][:, sc * 128 : (sc + 1) * 128]
                    nc.tensor.matmul(
                        yps[:, 0:512],
                        lhsT,
                        wo_sb[dc][:, 0:512],
                        start=(dc == 0),
                        stop=(dc == NDC - 1),
                    )
                    nc.tensor.matmul(
                        yps[:, 512:768],
                        lhsT,
                        wo_sb[dc][:, 512:768],
                        start=(dc == 0),
                        stop=(dc == NDC - 1),
                    )
                yt = y_pool.tile([128, D], F32, name="yt")
                nc.vector.tensor_add(yt, yps, bo_b)
                nc.sync.dma_start(y_d[sc * 128 : (sc + 1) * 128, :], yt)

    nc.compile()
    return nc


def _prep_inputs(x, Wq, bq, Wk, bk, Wv, bv, Wo, bo):
    """Host-side layout transforms + bf16 casts."""
    x = np.asarray(x)
    # xT per batch: [B, D, S] bf16
    xT = np.ascontiguousarray(x.transpose(0, 2, 1)).astype(_BF16)

    def pack_w(W):
        # W [H, D, DH] -> [NPAIR, 128(d_sub), D(dc*128+m)] where m in 0..127
        # indexes (head-in-pair, e): value[p, d_sub, dc*128+m] = W[2p + m//64, dc*128+d_sub, m%64]
        Wp = np.empty((NPAIR, 128, D), np.float32)
        W = np.asarray(W, np.float32)
        for p in range(NPAIR):
            blk = np.concatenate([W[2 * p], W[2 * p + 1]], axis=1)  # [D, 128]
            # want [d_sub, dc*128+m] = blk[dc*128+d_sub, m]
            Wp[p] = blk.reshape(NDC, 128, 128).transpose(1, 0, 2).reshape(128, D)
        return Wp.astype(_BF16)

    wq = pack_w(Wq)
    wk = pack_w(Wk)
    wv = pack_w(Wv)

    bqkv = np.empty((128, 18), np.float32)
    for j, b_ in enumerate((bq, bk, bv)):
        b_ = np.asarray(b_, np.float32)
        for p in range(NPAIR):
            bqkv[:, j * 6 + p] = np.concatenate([b_[2 * p], b_[2 * p + 1]])

    Wo = np.asarray(Wo, np.float32)
    wo = Wo.reshape(NDC, 128, D).astype(_BF16)

    bo_h = np.asarray(bo, np.float32).reshape(1, D)

    ident = np.zeros((128, 64), np.float32)
    ident[0:64] = np.eye(64)
    ident[64:128] = np.eye(64)
    ident = ident.astype(_BF16)

    shared = {
        "wq": wq,
        "wk": wk,
        "wv": wv,
        "wo": wo,
        "bqkv": bqkv,
        "bo": bo_h,
        "ident": ident,
    }
    return xT, shared


def kernel(x, Wq, bq, Wk, bk, Wv, bv, Wo, bo):
    from concourse.bass_utils import run_bass_kernel_spmd

    if "nc" not in _cache:
        _cache["nc"] = _build_program()
    nc = _cache["nc"]

    xT, shared = _prep_inputs(x, Wq, bq, Wk, bk, Wv, bv, Wo, bo)
    in_maps = [dict(shared, xT=np.ascontiguousarray(xT[b])) for b in range(B)]
    res = run_bass_kernel_spmd(nc, in_maps, core_ids=list(range(B)))
    y = np.stack([res.results[b]["y"] for b in range(B)], axis=0)
    return y.astype(np.float32)


# revision 22
# speedup vs baseline: 34.3957x; 34.3957x over previous
"""Multi-head attention (B=8, S=1024, D=768, H=12, DH=64) on 8 TRN2 NeuronCores.

Strategy: pure data parallelism over batch — core b computes batch element b
end-to-end (no collectives). Per core, activations are kept in transposed
[feature, token] layout so every matmul contracts over the partition dim:

  xT [768,1024]  (host-transposed, bf16)
  qT/kT/vT per head-pair [128,1024] = Wqkv_pair.T @ xT   (PE, k=128, m=128)
  S^T per (pair, tchunk, shalf) [128,1024] = (even|odd) scores, k=64 row-tiled
  P = exp(S^T * 0.125)        (ACT, psum->sbuf bf16)
  O^T[65,512] += [V|1].T @ P  (PE; row 64 = softmax denominator for free)
  normalize: recip(denom) -> DRAM -> partition-broadcast DMA -> DVE multiply
  Y [1024,768] = OT.T @ Wo + bo  (PE k=128; DVE bias-add; natural layout out)

All matmul inputs bf16 (fp32 PSUM accumulation); output f32.
"""

import sys

sys.path.insert(0, "/opt/trn_rl_repo")

import numpy as np
import ml_dtypes

B, S, D = 8, 1024, 768
H = 12
DH = 64
NPAIR = 6  # head pairs
NDC = 6  # 128-wide chunks of D
NTC = 8  # 128-wide chunks of S (key/t side)
NSC = 8  # 128-wide chunks of S (query/s side)

_BF16 = ml_dtypes.bfloat16

_cache = {}


def _build_program():
    import concourse.bass as bass
    import concourse.bacc as bacc
    import concourse.tile as tile
    from concourse import mybir

    F32 = mybir.dt.float32
    BF16 = mybir.dt.bfloat16
    Exp = mybir.ActivationFunctionType.Exp

    nc = bacc.Bacc("TRN2", target_bir_lowering=False, debug=False)

    # ---- DRAM I/O (per core) ----
    xT_d = nc.dram_tensor("xT", [D, S], BF16, kind="ExternalInput")
    wqkv_d = nc.dram_tensor("wqkv", [NPAIR, 128, 3 * D], BF16, kind="ExternalInput")
    wo_d = nc.dram_tensor("wo", [128, NDC * D], BF16, kind="ExternalInput")
    bqkv_d = nc.dram_tensor("bqkv", [128, 18], F32, kind="ExternalInput")
    bo_d = nc.dram_tensor("bo", [1, D], F32, kind="ExternalInput")
    ident_d = nc.dram_tensor("ident", [128, 64], BF16, kind="ExternalInput")
    y_d = nc.dram_tensor("y", [S, D], F32, kind="ExternalOutput")

    denom_d = nc.dram_tensor("denom_scr", [H, S], F32, kind="Internal")

    with tile.TileContext(nc) as tc:
        import contextlib

        ctx = contextlib.ExitStack()
        with ctx:
            const = ctx.enter_context(tc.tile_pool(name="const", bufs=1))
            wpool = ctx.enter_context(tc.tile_pool(name="wpool", bufs=1))
            qkv = ctx.enter_context(tc.tile_pool(name="qkv", bufs=3))
            vn_pool = ctx.enter_context(tc.tile_pool(name="vn", bufs=4))
            ot_pool = ctx.enter_context(tc.tile_pool(name="ot", bufs=1))
            e_pool = ctx.enter_context(tc.tile_pool(name="e", bufs=4))
            r_pool = ctx.enter_context(tc.tile_pool(name="r", bufs=4))
            y_pool = ctx.enter_context(tc.tile_pool(name="ysb", bufs=3))
            ps = ctx.enter_context(tc.tile_pool(name="ps", bufs=1, space="PSUM"))

            # ---- inputs to SBUF; critical-path first (xT + pair-0 weights) ----
            # xT resident tile; first projection's weights go first, then xT
            # chunks (fine-grained deps), then the rest.
            xt_all = wpool.tile([128, NDC * S], BF16, name="xt_all")
            xT = [xt_all[:, dc * S : (dc + 1) * S] for dc in range(NDC)]
            xt_src = xT_d.rearrange("(dc p) s -> p dc s", p=128)

            w_sb = {}
            wqkv_t = {}
            for p in range(NPAIR):
                wqkv_t[p] = wpool.tile([128, 3 * D], BF16, name=f"wqkv{p}")
                for i, wname in enumerate(("q", "k", "v")):
                    w_sb[wname, p] = wqkv_t[p][:, i * D : (i + 1) * D]

            nc.sync.dma_start(w_sb["q", 0], wqkv_d[0, :, 0:D])
            for dc in range(NDC):
                nc.sync.dma_start(xT[dc], xt_src[:, dc, :])
            nc.sync.dma_start(w_sb["k", 0], wqkv_d[0, :, D : 2 * D])
            nc.sync.dma_start(w_sb["v", 0], wqkv_d[0, :, 2 * D : 3 * D])
            bqkv = const.tile([128, 18], F32)
            nc.sync.dma_start(bqkv, bqkv_d[:, :])
            ident = const.tile([128, 64], BF16)
            nc.sync.dma_start(ident, ident_d[:, :])
            for p in range(1, NPAIR):
                nc.sync.dma_start(wqkv_t[p], wqkv_d[p, :, :])
            bo_b = const.tile([128, D], F32)
            nc.sync.dma_start(
                bo_b, bass.AP(tensor=bo_d, offset=0, ap=[[0, 128], [1, D]])
            )
            wo_all = wpool.tile([128, NDC * D], BF16, name="wo_all")
            nc.sync.dma_start(wo_all, wo_d[:, :])
            wo_sb = [wo_all[:, dc * D : (dc + 1) * D] for dc in range(NDC)]

            # persistent OT tiles (one per pair, [128, 1024] bf16)
            OT = [ot_pool.tile([128, S], BF16, name=f"OT{p}") for p in range(NPAIR)]

            def projection(p, wname, bias_col):
                """Compute (W_pair.T @ xT + b) -> bf16 [128, 1024] tile."""
                dst = qkv.tile([128, S], BF16, tag=wname, name=f"{wname}T{p}")
                w = w_sb[wname, p]
                for nh in range(2):
                    pst = ps.tile(
                        [128, 512], F32, tag="pp", bufs=2, name=f"pp_{wname}{p}{nh}"
                    )
                    for dc in range(NDC):
                        nc.tensor.matmul(
                            pst,
                            w[:, dc * 128 : (dc + 1) * 128],
                            xT[dc][:, nh * 512 : (nh + 1) * 512],
                            start=(dc == 0),
                            stop=(dc == NDC - 1),
                        )
                    # High priority: these evacs gate the next pair's scores;
                    # they must jump the DVE queue ahead of normalize work.
                    with tc.high_priority(offset=300):
                        nc.vector.tensor_scalar_add(
                            dst[:, nh * 512 : (nh + 1) * 512],
                            pst,
                            bqkv[:, bias_col : bias_col + 1],
                        )
                return dst

            def proj_and_vn(p):
                """Projections + V-natural transpose for pair p."""
                qT = projection(p, "q", 0 * 6 + p)
                kT = projection(p, "k", 1 * 6 + p)
                vT = projection(p, "v", 2 * 6 + p)
                vnat = []
                for h2 in range(2):
                    vps = ps.tile([128, 512], BF16, tag="pp", bufs=2, name=f"vn{p}_{h2}")
                    for tcb in range(NTC):
                        nc.tensor.transpose(
                            vps[:, tcb * 64 : (tcb + 1) * 64],
                            vT[h2 * 64 : (h2 + 1) * 64, tcb * 128 : (tcb + 1) * 128],
                            ident[h2 * 64 : (h2 + 1) * 64, :],
                        )
                    vn = vn_pool.tile([128, NTC * 65], BF16, name=f"vnat{p}_{h2}")
                    vn_r = vn.rearrange("a (b c) -> a b c", c=65)
                    nc.vector.tensor_copy(
                        vn_r[:, :, 0:64], vps.rearrange("a (b c) -> a b c", c=64)
                    )
                    nc.vector.memset(vn_r[:, :, 64:65], 1.0)
                    vnat.append(vn)
                return qT, kT, vnat

            def normalize(p, h2, ou_t, ssl, sh):
                """recip(denom) -> DRAM roundtrip -> partition-bcast -> mult."""
                h = 2 * p + h2
                n = ssl.stop - ssl.start
                rt = r_pool.tile([65, n], F32, tag="rt", bufs=2, name="rt")
                nc.vector.reciprocal(out=rt[64:65, :], in_=ou_t[64:65, ssl])
                nc.sync.dma_start(denom_d[h, ssl], rt[64:65, :])
                rb = r_pool.tile([64, n], F32, tag="rb", bufs=2, name="rb")
                nc.sync.dma_start(
                    rb,
                    bass.AP(
                        tensor=denom_d,
                        offset=h * S + ssl.start,
                        ap=[[0, 64], [1, n]],
                    ),
                )
                nc.vector.tensor_mul(
                    OT[p][h2 * 64 : (h2 + 1) * 64, ssl], ou_t[0:64, ssl], rb
                )

            for p in range(NPAIR):
                qT, kT, vnat = proj_and_vn(p)

                # attention core; s split in halves to fit PSUM
                ou = [
                    r_pool.tile([65, S], F32, tag="ou", bufs=3, name=f"ou{p}_{h2}")
                    for h2 in range(2)
                ]
                for sh in range(2):
                    ssl = slice(sh * 512, (sh + 1) * 512)
                    Ops = [
                        ps.tile([65, 512], F32, tag="o", bufs=2, name=f"o{p}_{sh}_{h2}")
                        for h2 in range(2)
                    ]
                    for tcb in range(NTC):
                        st = ps.tile([128, S], F32, tag="s", bufs=2, name=f"s{p}_{sh}_{tcb}")
                        nc.tensor.matmul(
                            st[:, 0:512],
                            kT[0:64, tcb * 128 : (tcb + 1) * 128],
                            qT[0:64, ssl],
                            start=True,
                            stop=True,
                        )
                        nc.tensor.matmul(
                            st[:, 512:1024],
                            kT[64:128, tcb * 128 : (tcb + 1) * 128],
                            qT[64:128, ssl],
                            start=True,
                            stop=True,
                        )
                        et = e_pool.tile([128, S], BF16, name="expS")
                        nc.scalar.activation(et, st, Exp, scale=0.125)
                        for h2 in range(2):
                            nc.tensor.matmul(
                                Ops[h2][:, :],
                                vnat[h2][:, tcb * 65 : (tcb + 1) * 65],
                                et[:, h2 * 512 : (h2 + 1) * 512],
                                start=(tcb == 0),
                                stop=(tcb == NTC - 1),
                            )
                    # evacuate unnormalized O (frees psum fast)
                    for h2 in range(2):
                        nc.vector.tensor_copy(ou[h2][:, ssl], Ops[h2])
                    if p == NPAIR - 1:
                        # last pair: normalize each s-half as soon as it's
                        # done, so the output projection isn't tail-blocked.
                        for h2 in range(2):
                            normalize(p, h2, ou[h2], ssl, sh)
                if p < NPAIR - 1:
                    # normalize per head, full width, off the critical path
                    for h2 in range(2):
                        normalize(p, h2, ou[h2], slice(0, S), None)

            # ---- output projection: Y[sc] = sum_dc OT[dc][:, sc].T @ Wo[dc] + bo
            for sc in range(NSC):
                yps = ps.tile([128, D], F32, tag="s", bufs=2, name=f"y{sc}")
                for dc in range(NDC):
                    lhsT = OT[dc][:, sc * 128 : (sc + 1) * 128]
                    nc.tensor.matmul(
                        yps[:, 0:512],
                        lhsT,
                        wo_sb[dc][:, 0:512],
                        start=(dc == 0),
                        stop=(dc == NDC - 1),
                    )
                    nc.tensor.matmul(
                        yps[:, 512:768],
                        lhsT,
                        wo_sb[dc][:, 512:768],
                        start=(dc == 0),
                        stop=(dc == NDC - 1),
                    )
                yt = y_pool.tile([128, D], F32, name="yt")
                nc.vector.tensor_add(yt, yps, bo_b)
                nc.sync.dma_start(y_d[sc * 128 : (sc + 1) * 128, :], yt)

    nc.compile()
    return nc


def _prep_inputs(x, Wq, bq, Wk, bk, Wv, bv, Wo, bo):
    """Host-side layout transforms + bf16 casts."""
    x = np.asarray(x)
    # xT per batch: [B, D, S] bf16
    xT = np.ascontiguousarray(x.transpose(0, 2, 1)).astype(_BF16)

    def pack_w(W):
        # W [H, D, DH] -> [NPAIR, 128(d_sub), D(dc*128+m)] where m in 0..127
        # indexes (head-in-pair, e): value[p, d_sub, dc*128+m] = W[2p + m//64, dc*128+d_sub, m%64]
        Wp = np.empty((NPAIR, 128, D), np.float32)
        W = np.asarray(W, np.float32)
        for p in range(NPAIR):
            blk = np.concatenate([W[2 * p], W[2 * p + 1]], axis=1)  # [D, 128]
            # want [d_sub, dc*128+m] = blk[dc*128+d_sub, m]
            Wp[p] = blk.reshape(NDC, 128, 128).transpose(1, 0, 2).reshape(128, D)
        return Wp

    # q|k|v blocks side by side: [NPAIR, 128, 3*768]
    wqkv = np.concatenate([pack_w(Wq), pack_w(Wk), pack_w(Wv)], axis=2).astype(_BF16)

    bqkv = np.empty((128, 18), np.float32)
    for j, b_ in enumerate((bq, bk, bv)):
        b_ = np.asarray(b_, np.float32)
        for p in range(NPAIR):
            bqkv[:, j * 6 + p] = np.concatenate([b_[2 * p], b_[2 * p + 1]])

    Wo = np.asarray(Wo, np.float32)
    # [128(d_sub), NDC*768]: wo[:, dc*768 + j] = Wo[dc*128 + d_sub, j]
    wo = Wo.reshape(NDC, 128, D).transpose(1, 0, 2).reshape(128, NDC * D).astype(_BF16)

    bo_h = np.asarray(bo, np.float32).reshape(1, D)

    ident = np.zeros((128, 64), np.float32)
    ident[0:64] = np.eye(64)
    ident[64:128] = np.eye(64)
    ident = ident.astype(_BF16)

    shared = {
        "wqkv": wqkv,
        "wo": wo,
        "bqkv": bqkv,
        "bo": bo_h,
        "ident": ident,
    }
    return xT, shared


def kernel(x, Wq, bq, Wk, bk, Wv, bv, Wo, bo):
    from concourse.bass_utils import run_bass_kernel_spmd

    if "nc" not in _cache:
        _cache["nc"] = _build_program()
    nc = _cache["nc"]

    xT, shared = _prep_inputs(x, Wq, bq, Wk, bk, Wv, bv, Wo, bo)
    in_maps = [dict(shared, xT=np.ascontiguousarray(xT[b])) for b in range(B)]
    res = run_bass_kernel_spmd(nc, in_maps, core_ids=list(range(B)))
    y = np.stack([res.results[b]["y"] for b in range(B)], axis=0)
    return y.astype(np.float32)
